# revision 2
# baseline (speedup 1.0000x reference)
"""DeformConv3D Trainium2 Bass kernel (raw-bass, 8-core SPMD), v2.

Shard: core -> (batch b, 16 z-planes). 65536 voxels/core, 512 j-columns
(voxel v = j*128 + p). Chunk = 16 j = 2048 voxels; 32 chunks/repeat.
Phase A (offset conv + field math) is pipelined LEAD units ahead of the
main loop; field math runs per quarter (128 j).

Data path per chunk:
  gather: 16 per-j indirect DMAs (verified [128,1]-offset semantics),
          1KB bf16 blocks from the 8-parity 2x2x2-blocked copy of x.
          Pool-engine SWDGE desc-gen (994ns/instr) is the bottleneck.
  combine (DVE): M = U (*) G  (bf16), fold dz, fold dy
  PE: 16 transposes R2 -> pT (bf16), 4 matmuls wstk @ scm -> pO (f32)
  exits: scm copies + out exits (f32->bf16) on ACT
  store: one 4KB/partition DMA per chunk, bf16; host adds bias + casts.

Offset conv in fp16 (xns, wofft); coords f32 with magic-number floor
(C = 3*2^22); U weights bf16.
"""

import numpy as np
import ml_dtypes

import concourse.bass as bass
import concourse.mybir as mybir
from concourse.bass import AP, IndirectOffsetOnAxis
from concourse.bass_utils import run_bass_kernel_spmd

bf16 = ml_dtypes.bfloat16
f32 = mybir.dt.float32
bft = mybir.dt.bfloat16
f16 = mybir.dt.float16
i8 = mybir.dt.int8
i32 = mybir.dt.int32
Alu = mybir.AluOpType

B, CIN, COUT, D, H, W = 2, 64, 128, 64, 64, 64
NCORE = 8
SH = D // (NCORE // B)       # 16 z-planes per core
NV = SH * H * W              # 65536 voxels per core
NJ = NV // 128               # 512 j-columns
BPS = B * 32 * 32 * 32       # blocks per parity selector
NBLK = 8 * BPS               # 524288
PADBLK = 64
K_CH = 16                    # j-columns per chunk
NCHUNK = NJ // K_CH          # 32
CHV = K_CH * 128             # 2048 voxels per chunk
LEAD = 16                    # phase-A lead (units)
NQ = 4                       # field-math quarters per repeat
QJ = NJ // NQ                # 128 j per quarter
MAGIC = float(3 * 2 ** 22)   # magic floor constant

_PROGRAM = None


def _build_program(repeat=1, debug_dump=False):
    nc = bass.Bass()

    xq_d = nc.declare_dram_parameter("xq", [NBLK + PADBLK, 512], bft, isOutput=False)
    xns_d = nc.declare_dram_parameter("xns", [CIN, NV], f16, isOutput=False)
    btile_d = nc.declare_dram_parameter("btile", [128, NJ * 3], f32, isOutput=False)
    rowb_d = nc.declare_dram_parameter("rowbase", [128, 1], f32, isOutput=False)
    wofft_d = nc.declare_dram_parameter("wofft", [64, 32], f16, isOutput=False)
    wstk_d = nc.declare_dram_parameter("wstack", [128, 128], bft, isOutput=False)
    ident_d = nc.declare_dram_parameter("ident", [128, 128], f32, isOutput=False)
    identb_d = nc.declare_dram_parameter("identb", [128, 128], bft, isOutput=False)
    out_d = nc.declare_dram_parameter("out", [COUT, NV], bft, isOutput=True)
    if debug_dump:
        dbgF_d = nc.declare_dram_parameter("dbgF", [128, NJ * 3], f32, isOutput=True)
        dbgI_d = nc.declare_dram_parameter("dbgI", [128, NJ], i32, isOutput=True)
        dbgU_d = nc.declare_dram_parameter("dbgU", [128, NJ * 8], bft, isOutput=True)
        dbgG_d = nc.declare_dram_parameter("dbgG", [128, K_CH * 512], bft, isOutput=True)
        dbgR_d = nc.declare_dram_parameter("dbgR", [128, K_CH * 128], bft, isOutput=True)
        dbgS_d = nc.declare_dram_parameter("dbgS", [128, CHV], bft, isOutput=True)

    ctxs = []

    def sb(name, shape, dtype):
        cm = nc.sbuf_tensor(name, shape, dtype)
        t = cm.__enter__()
        ctxs.append(cm)
        return t

    def ps(name, shape, dtype):
        cm = nc.psum_tensor(name, shape, dtype)
        t = cm.__enter__()
        ctxs.append(cm)
        return t

    def sem(name):
        cm = nc.semaphore(name)
        s = cm.__enter__()
        ctxs.append(cm)
        return s

    # constants
    btile = sb("sb_btile", [128, NJ * 3], f32)
    rowb = sb("sb_rowb", [128, 1], f32)
    wofft = sb("sb_wofft", [64, 32], f16)
    wstk = sb("sb_wstk", [128, 128], bft)
    ident = sb("sb_ident", [128, 128], f32)
    identb = sb("sb_identb", [128, 128], bft)
    # phase A
    xcm = [sb(f"sb_xcm{i}", [64, CHV], f16) for i in range(4)]
    stage = [sb(f"sb_stage{i}", [128, 512], f32) for i in range(2)]
    F = sb("sb_F", [128, NJ * 3], f32)
    # field-math temps (quarter-sized), baseline-proven op set
    TP = sb("sb_TP", [128, 3 * QJ], f32)
    TFr = sb("sb_TFr", [128, 3 * QJ], f32)
    TE3 = sb("sb_TE3", [128, 3 * QJ], f32)
    TIb = sb("sb_TIb", [128, 3 * QJ], i32)
    tA = sb("sb_tA", [128, QJ], f32)
    tB = sb("sb_tB", [128, QJ], f32)
    tC = sb("sb_tC", [128, QJ], f32)
    tD = sb("sb_tD", [128, QJ], f32)
    wz0 = sb("sb_wz0", [128, QJ], f32)
    wy0 = sb("sb_wy0", [128, QJ], f32)
    W4 = [sb(f"sb_W4_{k}", [128, QJ], f32) for k in range(4)]
    I = sb("sb_I", [128, NJ], i32)
    U = sb("sb_U", [128, NJ * 8], bft)
    # main loop
    G = [sb(f"sb_G{i}", [128, K_CH * 512], bft) for i in range(2)]
    M = sb("sb_M", [128, K_CH * 512], bft)
    R1 = sb("sb_R1", [128, K_CH * 256], bft)
    R2 = [sb(f"sb_R2_{i}", [128, K_CH * 128], bft) for i in range(2)]
    scm = [sb(f"sb_scm{i}", [128, CHV], bft) for i in range(2)]
    ost = [sb(f"sb_ost{i}", [128, CHV], bft) for i in range(2)]

    pofs = [ps(f"ps_pofs{i}", [128, 512], f32) for i in range(2)]
    ptr = [ps(f"ps_ptr{i}", [128, 128], f32) for i in range(2)]
    pT = [ps(f"ps_pT{i}", [128, 1024], bft) for i in range(2)]
    pO = [ps(f"ps_pO{i}", [128, 512], f32) for i in range(2)]

    s_ld = sem("s_ld")
    s_xcm = sem("s_xcm")
    s_offm = sem("s_offm")
    s_stg = sem("s_stg")
    s_trp = sem("s_trp")
    s_ext = sem("s_ext")
    s_fld = sem("s_fld")
    s_gth = sem("s_gth")
    s_mul = sem("s_mul")
    s_cmb = sem("s_cmb")
    s_trpS = sem("s_trpS")
    s_exS = sem("s_exS")
    s_mm = sem("s_mm")
    s_oex = sem("s_oex")
    s_out = sem("s_out")

    NCH = NCHUNK * repeat        # global chunks == global units
    NQG = NQ * repeat            # global quarters

    def wge(eng, s, n):
        if n > 0:
            eng.wait_ge(s, n)

    with nc.Block() as block:

        # ---------------- SP: HWDGE DMA ----------------
        @block.sync
        def _(sync):
            for dst, src in [(btile, btile_d), (rowb, rowb_d), (wofft, wofft_d),
                             (wstk, wstk_d), (ident, ident_d), (identb, identb_d)]:
                sync.dma_start(out=dst[:], in_=src[:]).then_inc(s_ld, 16)

            def load_xcm(u):
                ud = u % NCHUNK
                wge(sync, s_offm, 4 * (u - 3))
                sync.dma_start(
                    out=xcm[u % 4][:], in_=xns_d[:, CHV * ud : CHV * ud + CHV]
                ).then_inc(s_xcm, 16)

            for u in range(min(LEAD + 4, NCH)):
                load_xcm(u)
            for gcc in range(NCH):
                u = gcc + LEAD + 4
                if u < NCH:
                    load_xcm(u)
                ccd = gcc % NCHUNK
                wge(sync, s_oex, 4 * (gcc + 1))
                sync.dma_start(
                    out=out_d[:, CHV * ccd : CHV * ccd + CHV], in_=ost[gcc % 2][:]
                ).then_inc(s_out, 16)
            if debug_dump:
                sync.wait_ge(s_out, 16 * NCH)
                for dst, src in [(dbgF_d, F), (dbgI_d, I), (dbgU_d, U),
                                 (dbgG_d, G[0]), (dbgR_d, R2[0]), (dbgS_d, scm[0])]:
                    sync.dma_start(out=dst[:], in_=src[:]).then_inc(s_out, 16)

        # ---------------- PE ----------------
        @block.tensor
        def _(pe):
            wge(pe, s_ld, 96)

            def phase_a(u):
                for g in range(4):
                    wge(pe, s_xcm, 16 * (u + 1))
                    if g == 0:
                        wge(pe, s_stg, u - 1)   # pofs[u%2] free
                    nc.tensor.matmul(
                        out=pofs[u % 2][32 * g : 32 * g + 32, :],
                        lhsT=wofft[:],
                        rhs=xcm[u % 4][:, 512 * g : 512 * g + 512],
                        start=True,
                        stop=True,
                        tile_position=(0, 32 * g),
                    ).then_inc(s_offm, 1)
                for b in range(4):
                    k = 4 * u + b
                    wge(pe, s_stg, u + 1)
                    wge(pe, s_ext, k - 1)       # ptr[k%2] free
                    nc.tensor.transpose(
                        out=ptr[k % 2][:],
                        in_=stage[u % 2][:, 128 * b : 128 * b + 128],
                        identity=ident[:],
                    ).then_inc(s_trp, 1)

            for u in range(min(LEAD, NCH)):
                phase_a(u)
            for gcc in range(NCH):
                if gcc + LEAD < NCH:
                    phase_a(gcc + LEAD)
                # S-transposes
                for k in range(16):
                    h = k // 8
                    wge(pe, s_cmb, gcc + 1)
                    wge(pe, s_exS, 2 * (gcc - 1) + h + 1)   # pT[h] free
                    nc.tensor.transpose(
                        out=pT[h][:, 128 * (k % 8) : 128 * (k % 8) + 128],
                        in_=R2[gcc % 2][:, 128 * k : 128 * k + 128],
                        identity=identb[:],
                    ).then_inc(s_trpS, 1)
                # main matmuls
                for m in range(4):
                    T = 4 * gcc + m
                    wge(pe, s_exS, 2 * gcc + (m // 2) + 1)
                    wge(pe, s_oex, T - 1)  # pO[m%2] free: exit T-2 done
                    nc.tensor.matmul(
                        out=pO[m % 2][:],
                        lhsT=wstk[:],
                        rhs=scm[gcc % 2][:, 512 * m : 512 * m + 512],
                        start=True,
                        stop=True,
                    ).then_inc(s_mm, 1)

        # ---------------- ACT ----------------
        @block.scalar
        def _(act):
            def phase_a(u):
                ud = u % NCHUNK
                wge(act, s_offm, 4 * (u + 1))
                wge(act, s_trp, 4 * (u - 1))     # stage[u%2] free
                nc.scalar.copy(out=stage[u % 2][:], in_=pofs[u % 2][:]).then_inc(
                    s_stg, 1
                )
                for b in range(4):
                    k = 4 * u + b
                    wge(act, s_trp, k + 1)
                    src = ptr[k % 2][:].rearrange("p (g r) -> p g r", r=32)[:, :, 0:3]
                    col = 48 * ud + 3 * b
                    fap = F[:]
                    dst = AP(fap.tensor, fap.offset + col, [fap.ap[0], [12, 4], [1, 3]])
                    nc.scalar.copy(out=dst, in_=src).then_inc(s_ext, 1)

            for u in range(min(LEAD, NCH)):
                phase_a(u)
            for gcc in range(NCH):
                if gcc + LEAD < NCH:
                    phase_a(gcc + LEAD)
                for h in range(2):
                    wge(act, s_trpS, 16 * gcc + 8 * (h + 1))
                    wge(act, s_mm, 4 * (gcc - 1))   # scm[gcc%2] free
                    nc.scalar.copy(
                        out=scm[gcc % 2][:, 1024 * h : 1024 * h + 1024], in_=pT[h][:]
                    ).then_inc(s_exS, 1)
                for m in range(4):
                    T = 4 * gcc + m
                    wge(act, s_mm, T + 1)
                    wge(act, s_out, 16 * (gcc - 1))  # ost[gcc%2] free
                    nc.scalar.copy(
                        out=ost[gcc % 2][:, 512 * m : 512 * m + 512], in_=pO[m % 2][:]
                    ).then_inc(s_oex, 1)

        # ---------------- POOL: per-j gathers only ----------------
        @block.gpsimd
        def _(pool):
            for gcc in range(NCH):
                ccd = gcc % NCHUNK
                wge(pool, s_fld, gcc // 8 + 1)
                wge(pool, s_mul, gcc - 1)        # G[gcc%2] free
                for jj in range(K_CH):
                    j = K_CH * ccd + jj
                    pool.indirect_dma_start(
                        out=G[gcc % 2][:, 512 * jj : 512 * jj + 512],
                        out_offset=None,
                        in_=xq_d[:],
                        in_offset=IndirectOffsetOnAxis(ap=I[:, j : j + 1], axis=0),
                    ).then_inc(s_gth, 16)

        # ---------------- DVE ----------------
        @block.vector
        def _(dve):
            v = nc.vector

            def comp(tile, c):
                return tile[:].rearrange("p (j c) -> p j c", c=3)[:, :, c]

            def field(gq):
                qd = gq % NQ
                c3 = 3 * QJ * qd
                cq = QJ * qd
                cu = 8 * QJ * qd
                wge(dve, s_ext, 32 * (gq + 1))
                Fq = F[:, c3 : c3 + 3 * QJ]
                Bq = btile[:, c3 : c3 + 3 * QJ]
                P, Fr, tE3, Ibig = TP, TFr, TE3, TIb
                v.tensor_add(out=P[:], in0=Fq, in1=Bq)
                v.tensor_scalar(out=P[:], in0=P[:], scalar1=0.0, scalar2=63.0,
                                op0=Alu.max, op1=Alu.min)
                # floor via i32 round-trip + is_gt fixup
                v.tensor_copy(out=Ibig[:], in_=P[:])
                v.tensor_copy(out=Fr[:], in_=Ibig[:])
                v.tensor_tensor(out=tE3[:], in0=Fr[:], in1=P[:], op=Alu.is_gt)
                v.tensor_sub(out=Fr[:], in0=Fr[:], in1=tE3[:])   # Fr = floor(P)
                v.tensor_sub(out=P[:], in0=P[:], in1=Fr[:])      # P = frac
                v.tensor_copy(out=tE3[:], in_=Fr[:])
                v.tensor_copy(out=Fr[:], in_=P[:])               # Fr = frac
                v.tensor_copy(out=P[:], in_=tE3[:])              # P = floor

                ix0, iy0, iz0 = comp(P, 0), comp(P, 1), comp(P, 2)
                fx, fy, fz = comp(Fr, 0), comp(Fr, 1), comp(Fr, 2)
                Ism = Ibig[:].rearrange("p (j c) -> p j c", c=3)[:, :, 0]
                tE1 = tE3[:].rearrange("p (j c) -> p j c", c=3)[:, :, 0]

                def halve(coord, Zf_out, h_out):
                    v.tensor_scalar(out=tD[:], in0=coord, scalar1=0.5,
                                    scalar2=None, op0=Alu.mult)
                    v.tensor_copy(out=Ism, in_=tD[:])
                    v.tensor_copy(out=Zf_out, in_=Ism)
                    v.tensor_tensor(out=tE1, in0=Zf_out, in1=tD[:], op=Alu.is_gt)
                    v.tensor_sub(out=Zf_out, in0=Zf_out, in1=tE1)
                    v.tensor_scalar(out=h_out, in0=Zf_out, scalar1=-2.0,
                                    scalar2=None, op0=Alu.mult)
                    v.tensor_add(out=h_out, in0=h_out, in1=coord)

                halve(iz0, tC[:], tB[:])
                v.tensor_scalar(out=tA[:], in0=tB[:], scalar1=262144.0,
                                scalar2=None, op0=Alu.mult)
                v.tensor_scalar(out=tC[:], in0=tC[:], scalar1=1024.0,
                                scalar2=None, op0=Alu.mult)
                v.tensor_add(out=tA[:], in0=tA[:], in1=tC[:])
                halve(iy0, tC[:], tB[:])
                v.tensor_scalar(out=tB[:], in0=tB[:], scalar1=131072.0,
                                scalar2=None, op0=Alu.mult)
                v.tensor_add(out=tA[:], in0=tA[:], in1=tB[:])
                v.tensor_scalar(out=tC[:], in0=tC[:], scalar1=32.0,
                                scalar2=None, op0=Alu.mult)
                v.tensor_add(out=tA[:], in0=tA[:], in1=tC[:])
                halve(ix0, tC[:], tB[:])
                v.tensor_scalar(out=tB[:], in0=tB[:], scalar1=65536.0,
                                scalar2=None, op0=Alu.mult)
                v.tensor_add(out=tA[:], in0=tA[:], in1=tB[:])
                v.tensor_add(out=tA[:], in0=tA[:], in1=tC[:])
                v.tensor_scalar(out=tA[:], in0=tA[:], scalar1=rowb[:, 0:1],
                                scalar2=None, op0=Alu.add)
                v.tensor_copy(out=I[:, cq : cq + QJ], in_=tA[:])

                v.tensor_scalar(out=wz0[:], in0=fz, scalar1=-1.0, scalar2=1.0,
                                op0=Alu.mult, op1=Alu.add)
                v.tensor_scalar(out=wy0[:], in0=fy, scalar1=-1.0, scalar2=1.0,
                                op0=Alu.mult, op1=Alu.add)
                v.tensor_mul(out=W4[0][:], in0=wz0[:], in1=wy0[:])
                v.tensor_sub(out=W4[1][:], in0=wz0[:], in1=W4[0][:])
                v.tensor_sub(out=W4[2][:], in0=wy0[:], in1=W4[0][:])
                v.tensor_sub(out=W4[3][:], in0=fz, in1=W4[2][:])
                Uv = U[:, cu : cu + 8 * QJ].rearrange("p (j s) -> p j s", s=8)
                last = None
                for k, (dz, dy) in enumerate([(0, 0), (0, 1), (1, 0), (1, 1)]):
                    s = 4 * dz + 2 * dy
                    v.tensor_mul(out=tB[:], in0=W4[k][:], in1=fx)
                    v.tensor_sub(out=tC[:], in0=W4[k][:], in1=tB[:])
                    v.tensor_copy(out=Uv[:, :, s + 1], in_=tB[:])
                    last = v.tensor_copy(out=Uv[:, :, s], in_=tC[:])
                last.then_inc(s_fld, 1)

            wge(dve, s_ld, 96)
            field(0)
            for gcc in range(NCH):
                if gcc % 8 == 4 and gcc // 8 + 1 < NQG:
                    field(gcc // 8 + 1)
                ccd = gcc % NCHUNK
                # combine
                wge(dve, s_gth, 16 * K_CH * (gcc + 1))
                uslice = U[:, 8 * K_CH * ccd : 8 * K_CH * ccd + 8 * K_CH]
                ub = AP(uslice.tensor, uslice.offset, uslice.ap + [[0, 64]])
                gv = G[gcc % 2][:].rearrange("p (js c) -> p js c", c=64)
                mv = M[:].rearrange("p (js c) -> p js c", c=64)
                v.tensor_tensor(out=mv, in0=gv, in1=ub, op=Alu.mult).then_inc(s_mul, 1)
                wge(dve, s_trpS, 16 * (gcc - 1))  # R2[gcc%2] free
                m5 = M[:].rearrange("p (j dz r) -> p j dz r", dz=2, r=256)
                v.tensor_add(
                    out=R1[:].rearrange("p (j r) -> p j r", r=256),
                    in0=m5[:, :, 0, :], in1=m5[:, :, 1, :])
                r5 = R1[:].rearrange("p (j dy r) -> p j dy r", dy=2, r=128)
                v.tensor_add(
                    out=R2[gcc % 2][:].rearrange("p (j r) -> p j r", r=128),
                    in0=r5[:, :, 0, :], in1=r5[:, :, 1, :]).then_inc(s_cmb, 1)

    for cm in reversed(ctxs):
        cm.__exit__(None, None, None)
    return nc


def _get_program():
    global _PROGRAM
    if _PROGRAM is None:
        _PROGRAM = _build_program()
    return _PROGRAM


def build_bench(repeat):
    return _build_program(repeat=repeat)


def _prep_inputs(x, w_off, b_off, w_conv, b_conv):
    x = np.ascontiguousarray(np.asarray(x, np.float32))
    w_off = np.asarray(w_off, np.float32)
    b_off = np.asarray(b_off, np.float32)
    w_conv = np.asarray(w_conv, np.float32)
    b_conv = np.asarray(b_conv, np.float32)

    xb = x.transpose(0, 2, 3, 4, 1).astype(bf16)  # [B, D, H, W, C]
    xpad = np.zeros((B, D + 2, H + 2, W + 2, CIN), bf16)
    xpad[:, :D, :H, :W] = xb
    xq = np.zeros((NBLK + PADBLK, 512), bf16)
    for sel in range(8):
        pz, py, px = (sel >> 2) & 1, (sel >> 1) & 1, sel & 1
        v = xpad[:, pz : pz + 64, py : py + 64, px : px + 64, :]
        v = v.reshape(B, 32, 2, 32, 2, 32, 2, CIN)
        v = v.transpose(0, 1, 3, 5, 2, 4, 6, 7)  # B,Z,Y,X,dz,dy,dx,C
        xq[sel * BPS : (sel + 1) * BPS] = v.reshape(BPS, 512)

    wofft = np.zeros((64, 32), np.float16)
    wofft[:, :3] = (w_off * 32.0).T.astype(np.float16)
    wstack = np.concatenate([w_conv.T, w_conv.T], axis=0).astype(bf16)
    ident = np.eye(128, dtype=np.float32)
    identb = ident.astype(bf16)

    in_maps = []
    for core in range(NCORE):
        b = core // (NCORE // B)
        z0 = (core % (NCORE // B)) * SH
        xns = np.ascontiguousarray(
            x[b, :, z0 : z0 + SH].reshape(CIN, NV).astype(np.float16)
        )
        v = np.arange(NV)
        zz = z0 + v // (H * W)
        yy = (v // W) % H
        xx = v % W
        base = np.stack(
            [
                64.0 * xx / 63.0 - 0.5 + 32.0 * b_off[0],
                64.0 * yy / 63.0 - 0.5 + 32.0 * b_off[1],
                64.0 * zz / 63.0 - 0.5 + 32.0 * b_off[2],
            ],
            axis=1,
        ).astype(np.float32)
        btile = np.ascontiguousarray(
            base.reshape(NJ, 128, 3).transpose(1, 0, 2).reshape(128, NJ * 3)
        )
        rowbase = np.full((128, 1), b * 32768.0, np.float32)
        in_maps.append(
            {
                "xq": xq,
                "xns": xns,
                "btile": btile,
                "rowbase": rowbase,
                "wofft": wofft,
                "wstack": wstack,
                "ident": ident,
                "identb": identb,
            }
        )
    return in_maps


def _assemble(results, b_conv):
    out = np.zeros((B, COUT, D, H, W), np.float32)
    for core in range(NCORE):
        b = core // (NCORE // B)
        z0 = (core % (NCORE // B)) * SH
        out[b, :, z0 : z0 + SH] = (
            results[core]["out"].astype(np.float32).reshape(COUT, SH, H, W)
        )
    out += np.asarray(b_conv, np.float32)[None, :, None, None, None]
    return out


def kernel(x, w_off, b_off, w_conv, b_conv):
    nc = _get_program()
    in_maps = _prep_inputs(x, w_off, b_off, w_conv, b_conv)
    res = run_bass_kernel_spmd(nc, in_maps, list(range(NCORE)))
    return _assemble(res.results, b_conv)
